# revision 20
# baseline (speedup 1.0000x reference)
"""Trainium2 Bass kernel: fp8-emulated attention, 20 heads x 4096 x 64.

Sharding: flattened (head, q) rows split evenly across 8 cores ->
2.5 heads per core (2 full-head segments + 1 half-head segment each,
identical SPMD graph; per-core in_maps differ only in data).

Per-core algorithm (S.T layout flash-style, no max subtraction -- scores
are bounded ~|s|<7 so fp32 exp never overflows):
  q8 = fp8(q) replicated on both partition halves, d-major [128, 10240]
  k8 = fp8(k) pair-packed [128, 16*128] per head (even kv-blocks on
       partitions 0-63, odd on 64-127) -> row-packed (tile_position)
       pairs of QK^T matmuls, K=64 contraction each.
  S.T block [128 kv, 512 q] in PSUM -> exp via ScalarE (exact, scale=1/8
  fused) or VectorE (Schraudolph int16 bit-trick -> bf16) -> P.T bf16.
  PV: O.T[65, 512] += [V_blk | ones].T @ P.T_blk accumulated over 32
  kv-blocks in PSUM; row 64 = softmax denominators.
  Epilogue: copy to SBUF, PE-transpose [65,128] tiles -> [128,65],
  reciprocal + per-partition scale -> out rows [q, 64] -> DMA.
"""

import os

import numpy as np

import concourse.bass as bass
import concourse.tile as tile
from concourse import bacc, mybir
from concourse.bass import ts
from concourse.bass_utils import run_bass_kernel_spmd
from concourse.masks import make_identity

B, S, D = 20, 4096, 64
NCORES = 8
ROWS_PER_CORE = B * S // NCORES  # 10240
HALF = S // 2  # 2048
NQ = 512  # q columns per chunk (one PSUM bank of fp32)
NPAIR = 16  # kv-block pairs per head (32 blocks of 128)

F32 = mybir.dt.float32
F8 = mybir.dt.float8e4
BF16 = mybir.dt.bfloat16
I16 = mybir.dt.int16

# Schraudolph exp constants for bf16 bit-trick: i16 = A*s + Bc, bitcast bf16
# exp(s/8) = 2^(s * 0.125 * log2(e)); bf16: i = 128*(log2(y) + 127)
SCH_A = 0.125 * 1.4426950408889634 * 128.0
SCH_B = 127.0 * 128.0 - 128.0 * 0.0579

# Fraction control: pair indices where the DVE computes the pair's whole
# [128, 1024] exp tile (Schraudolph); other pairs go to ScalarE (exact).
# DVE share = len(DVE_PAIRS)/16 of all exp work.
DVE_PAIRS = frozenset()  # fallback: all exp on ScalarE
if os.environ.get("KERNEL_DVE_EXP", "1") == "1":
    DVE_PAIRS = frozenset({0, 2, 4, 6, 8, 10})  # 37.5%

LAST_EXEC_TIME_NS = None
LAST_RESULTS = None

_CACHED = None


def _core_segments(core):
    """Returns (headA, headB, (headC, qoff)) for this core."""
    start = core * ROWS_PER_CORE
    h = start // S
    if core % 2 == 0:
        return h, h + 1, (h + 2, 0)
    else:
        return h + 1, h + 2, (h, HALF)


def _build_graph(
    rows=ROWS_PER_CORE,
    npair=NPAIR,
    segs=None,
    nheads=3,
    num_devices=NCORES,
    dve_pairs=None,
):
    """segs: list of (head_slot, q_row_base, n_q_rows)."""
    if segs is None:
        segs = [(0, 0, S), (1, S, S), (2, 2 * S, HALF)]
    if dve_pairs is None:
        dve_pairs = DVE_PAIRS
    nc = bacc.Bacc(
        "TRN2",
        target_bir_lowering=False,
        debug=False,
        num_devices=num_devices,
    )
    qT_ext = nc.dram_tensor("qT", [64, rows], F32, kind="ExternalInput").ap()
    kp_ext = nc.dram_tensor(
        "kp", [nheads, 128, npair * 128], F32, kind="ExternalInput"
    ).ap()
    vp_ext = nc.dram_tensor(
        "vp", [nheads, 128, 2 * npair * 65], F32, kind="ExternalInput"
    ).ap()
    out_ext = nc.dram_tensor("out", [rows, 64], F32, kind="ExternalOutput").ap()

    KW = npair * 128  # k columns per head
    VW = 2 * npair * 65  # v columns per head

    with tile.TileContext(nc) as tc:
        with (
            tc.tile_pool(name="persist", bufs=1) as persist,
            tc.tile_pool(name="stage", bufs=2) as stage,
            tc.tile_pool(name="pbuf", bufs=6) as pbuf,
            tc.tile_pool(name="work", bufs=2) as work,
            tc.tile_pool(name="qkpsum", bufs=2, space="PSUM") as qkpsum,
            tc.tile_pool(name="otpsum", bufs=2, space="PSUM") as otpsum,
        ):
            # ---- persistent operand tiles (split per head / per q-chunk
            # so the first segment's compute can start while later
            # heads are still loading) ----
            qc = min(2048, rows)
            nqc = rows // qc
            ident = persist.tile([65, 65], F32)
            make_identity(nc, ident[:])

            q8_t = [
                persist.tile([128, qc], F8, name=f"q8_{c}") for c in range(nqc)
            ]
            k8_t = [
                persist.tile([128, KW], F8, name=f"k8_{h}") for h in range(nheads)
            ]
            v8b_t = [
                persist.tile([128, VW], BF16, name=f"v8b_{h}")
                for h in range(nheads)
            ]

            def load_head(h):
                st = stage.tile([128, 2080], F32, tag="stage", name=f"stk{h}")
                nc.sync.dma_start(st[:, 0:KW], kp_ext[h])
                nc.any.tensor_copy(k8_t[h][:], st[:, 0:KW])
                st = stage.tile([128, 2080], F32, tag="stage", name=f"stv{h}")
                nc.sync.dma_start(st[:, 0:VW], vp_ext[h])
                v8f = work.tile([128, VW], F8, tag="v8f", name=f"v8f{h}")
                nc.any.tensor_copy(v8f[:], st[:, 0:VW])
                nc.any.tensor_copy(v8b_t[h][:], v8f[:])

            def load_q(c):
                st = stage.tile([128, 2080], F32, tag="stage", name=f"stq{c}")
                nc.sync.dma_start(st[0:64, 0:qc], qT_ext[:, ts(c, qc)])
                nc.sync.dma_start(st[64:128, 0:qc], qT_ext[:, ts(c, qc)])
                nc.any.tensor_copy(q8_t[c][:], st[:, 0:qc])

            load_head(0)
            load_q(0)
            for h in range(1, nheads):
                load_head(h)
            for c in range(1, nqc):
                load_q(c)

            # ---- main attention loops (software-pipelined) ----
            # Flat chunk list across segments: (head_slot, qtile, qo, qoff)
            chunks = []
            for slot, qbase, nq in segs:
                for chunk in range(nq // NQ):
                    qoff = qbase + chunk * NQ
                    chunks.append((slot, q8_t[qoff // qc], qoff % qc, qoff))

            def emit_qk_a(slot, qtile, qo, p):
                # QK^T row-packed pair: A on partitions 0-63, B on
                # 64-127 (tile_position auto-derived from base partition)
                qk = qkpsum.tile(
                    [128, 2 * NQ], F32, tag="qk", bufs=3, name="qk"
                )
                kA = k8_t[slot][0:64, p * 128 : (p + 1) * 128]
                nc.tensor.matmul(
                    qk[:, 0:NQ], kA, qtile[0:64, qo : qo + NQ],
                    start=True, stop=True,
                )
                return qk

            def emit_qk_b(slot, qtile, qo, p, qk):
                kB = k8_t[slot][64:128, p * 128 : (p + 1) * 128]
                nc.tensor.matmul(
                    qk[:, NQ : 2 * NQ], kB, qtile[64:128, qo : qo + NQ],
                    start=True, stop=True,
                )

            def emit_exp(qk, p):
                # exp of the whole pair tile [128, 2*NQ] in ONE op on one
                # engine (halves the fixed per-op overhead + sem count)
                pab = pbuf.tile([128, 2 * NQ], BF16, tag="p", name="pab")
                if p in dve_pairs:
                    nc.vector.tensor_scalar(
                        pab[:].bitcast(I16), qk[:],
                        SCH_A, SCH_B,
                        mybir.AluOpType.mult, mybir.AluOpType.add,
                    )
                else:
                    nc.scalar.activation(
                        pab[:], qk[:],
                        mybir.ActivationFunctionType.Exp, scale=0.125,
                    )
                return pab

            def emit_pv(slot, ot, pab, p, which):
                v = v8b_t[slot][
                    :, (2 * p + which) * 65 : (2 * p + which + 1) * 65
                ]
                nc.tensor.matmul(
                    ot[:], v, pab[:, which * NQ : (which + 1) * NQ],
                    start=(p == 0 and which == 0),
                    stop=(p == npair - 1 and which == 1),
                    skip_group_check=True,
                )

            def make_epilogue(ot, qoff):
                def epi():
                    ot_sb = work.tile([65, NQ], F32, tag="otsb", name="ot_sb")
                    nc.vector.tensor_copy(ot_sb[:], ot[:])
                    osb = work.tile([128, 4 * 64], F32, tag="osb", name="osb")
                    for t in range(4):
                        tr = otpsum.tile(
                            [128, 65], F32, tag="ot", bufs=2, name=f"tr{t}"
                        )
                        nc.tensor.transpose(tr[:], ot_sb[:, ts(t, 128)], ident[:])
                        rc = work.tile([128, 1], F32, tag="rc", name="rc")
                        nc.vector.reciprocal(rc[:], tr[:, 64:65])
                        nc.vector.tensor_scalar(
                            osb[:, ts(t, 64)], tr[:, 0:64],
                            rc[:], None, mybir.AluOpType.mult,
                        )
                    nc.sync.dma_start(
                        out_ext[qoff : qoff + NQ, :].rearrange(
                            "(b p) d -> p b d", p=128
                        ),
                        osb[:].rearrange("p (b d) -> p b d", d=64),
                    )

                return epi

            def emit_qk_pair(slot, qtile, qo, p):
                qk = emit_qk_a(slot, qtile, qo, p)
                emit_qk_b(slot, qtile, qo, p, qk)
                return qk

            pending_epi = None
            for slot, qtile, qo, qoff in chunks:
                ot = otpsum.tile([65, NQ], F32, tag="ot", bufs=2, name="ot")
                qks = {0: emit_qk_pair(slot, qtile, qo, 0)}
                if npair > 1:
                    qks[1] = emit_qk_pair(slot, qtile, qo, 1)
                for p in range(npair):
                    pab = emit_exp(qks.pop(p), p)
                    # PE order: interleave the prefetch QK pair with the
                    # two PV accumulates so each PV's drain is hidden by
                    # an independent matmul, and exp(p) has ~3 windows of
                    # cover before PV(p) issues.
                    if p + 2 < npair:
                        qk2 = emit_qk_a(slot, qtile, qo, p + 2)
                        emit_pv(slot, ot, pab, p, 0)
                        emit_qk_b(slot, qtile, qo, p + 2, qk2)
                        emit_pv(slot, ot, pab, p, 1)
                        qks[p + 2] = qk2
                    else:
                        emit_pv(slot, ot, pab, p, 0)
                        emit_pv(slot, ot, pab, p, 1)
                    if p == 1 and pending_epi is not None:
                        pending_epi()
                        pending_epi = None
                if pending_epi is not None:
                    pending_epi()
                pending_epi = make_epilogue(ot, qoff)
            pending_epi()

    nc.compile()
    return nc


def _prep_core_inputs(core, q, k, v):
    hA, hB, (hC, qoff) = _core_segments(core)
    qT = np.empty((64, ROWS_PER_CORE), np.float32)
    qT[:, 0:S] = q[hA].T
    qT[:, S : 2 * S] = q[hB].T
    qT[:, 2 * S :] = q[hC, qoff : qoff + HALF].T

    kp = np.empty((3, 128, NPAIR * 128), np.float32)
    vp = np.empty((3, 128, 32 * 65), np.float32)
    for slot, h in enumerate((hA, hB, hC)):
        kt = np.ascontiguousarray(k[h].T).reshape(64, 32, 128)
        kp[slot, 0:64] = kt[:, 0::2, :].reshape(64, NPAIR * 128)
        kp[slot, 64:128] = kt[:, 1::2, :].reshape(64, NPAIR * 128)
        vb = v[h].reshape(32, 128, 64).transpose(1, 0, 2)  # [128, 32, 64]
        vpk = np.concatenate(
            [vb, np.ones((128, 32, 1), np.float32)], axis=2
        )  # [128, 32, 65]
        vp[slot] = vpk.reshape(128, 32 * 65)
    return {"qT": np.ascontiguousarray(qT), "kp": kp, "vp": vp}


def kernel(q, k, v):
    global LAST_EXEC_TIME_NS, LAST_RESULTS, _CACHED
    q = np.asarray(q, np.float32)
    k = np.asarray(k, np.float32)
    v = np.asarray(v, np.float32)

    if _CACHED is None:
        _CACHED = _build_graph()
    nc = _CACHED

    in_maps = [_prep_core_inputs(i, q, k, v) for i in range(NCORES)]

    trace = os.environ.get("KERNEL_TRACE", "0") == "1"
    kwargs = {}
    if trace:
        kwargs = dict(trace=True, trace_cores=[0])
    res = run_bass_kernel_spmd(nc, in_maps, core_ids=list(range(NCORES)), **kwargs)
    LAST_RESULTS = res
    LAST_EXEC_TIME_NS = res.exec_time_ns

    out = np.empty((B, S, D), np.float32)
    for core in range(NCORES):
        o = res.results[core]["out"]
        hA, hB, (hC, qoff) = _core_segments(core)
        out[hA] = o[0:S]
        out[hB] = o[S : 2 * S]
        out[hC, qoff : qoff + HALF] = o[2 * S :]
    return out


# revision 21
# speedup vs baseline: 1.2287x; 1.2287x over previous
"""Trainium2 Bass kernel: fp8-emulated attention, 20 heads x 4096 x 64.

Sharding: flattened (head, q) rows split evenly across 8 cores ->
2.5 heads per core (2 full-head segments + 1 half-head segment each,
identical SPMD graph; per-core in_maps differ only in data).

Per-core algorithm (S.T layout flash-style, no max subtraction -- scores
are bounded ~|s|<7 so fp32 exp never overflows):
  q8 = fp8(q) replicated on both partition halves, d-major [128, 10240]
  k8 = fp8(k) pair-packed [128, 16*128] per head (even kv-blocks on
       partitions 0-63, odd on 64-127) -> row-packed (tile_position)
       pairs of QK^T matmuls, K=64 contraction each.
  S.T block [128 kv, 512 q] in PSUM -> exp via ScalarE (exact, scale=1/8
  fused) or VectorE (Schraudolph int16 bit-trick -> bf16) -> P.T bf16.
  PV: O.T[65, 512] += [V_blk | ones].T @ P.T_blk accumulated over 32
  kv-blocks in PSUM; row 64 = softmax denominators.
  Epilogue: copy to SBUF, PE-transpose [65,128] tiles -> [128,65],
  reciprocal + per-partition scale -> out rows [q, 64] -> DMA.
"""

import os

import numpy as np

import concourse.bass as bass
import concourse.tile as tile
from concourse import bacc, mybir
from concourse.bass import ts
from concourse.bass_utils import run_bass_kernel_spmd
from concourse.masks import make_identity

B, S, D = 20, 4096, 64
NCORES = 8
ROWS_PER_CORE = B * S // NCORES  # 10240
HALF = S // 2  # 2048
NQ = 512  # q columns per chunk (one PSUM bank of fp32)
NPAIR = 16  # kv-block pairs per head (32 blocks of 128)

F32 = mybir.dt.float32
F8 = mybir.dt.float8e4
BF16 = mybir.dt.bfloat16
I16 = mybir.dt.int16

# Schraudolph exp constants for bf16 bit-trick: i16 = A*s + Bc, bitcast bf16
# exp(s/8) = 2^(s * 0.125 * log2(e)); bf16: i = 128*(log2(y) + 127)
SCH_A = 0.125 * 1.4426950408889634 * 128.0
SCH_B = 127.0 * 128.0 - 128.0 * 0.0579

# Fraction control: pair indices where the DVE computes the pair's whole
# [128, 1024] exp tile (Schraudolph); other pairs go to ScalarE (exact).
# DVE share = len(DVE_PAIRS)/16 of all exp work.
DVE_PAIRS = frozenset()  # fallback: all exp on ScalarE
if os.environ.get("KERNEL_DVE_EXP", "1") == "1":
    DVE_PAIRS = frozenset({0, 2, 4, 6, 8, 10})  # 37.5%

LAST_EXEC_TIME_NS = None
LAST_RESULTS = None

_CACHED = None


def _core_segments(core):
    """Returns (headA, headB, (headC, qoff)) for this core."""
    start = core * ROWS_PER_CORE
    h = start // S
    if core % 2 == 0:
        return h, h + 1, (h + 2, 0)
    else:
        return h + 1, h + 2, (h, HALF)


def _build_graph(
    rows=ROWS_PER_CORE,
    npair=NPAIR,
    segs=None,
    nheads=3,
    num_devices=NCORES,
    dve_pairs=None,
):
    """segs: list of (head_slot, q_row_base, n_q_rows)."""
    if segs is None:
        segs = [(0, 0, S), (1, S, S), (2, 2 * S, HALF)]
    if dve_pairs is None:
        dve_pairs = DVE_PAIRS
    nc = bacc.Bacc(
        "TRN2",
        target_bir_lowering=False,
        debug=False,
        num_devices=num_devices,
    )
    qT_ext = nc.dram_tensor("qT", [64, rows], F32, kind="ExternalInput").ap()
    kp_ext = nc.dram_tensor(
        "kp", [nheads, 128, npair * 128], F32, kind="ExternalInput"
    ).ap()
    vp_ext = nc.dram_tensor(
        "vp", [nheads, 128, 2 * npair * 65], F32, kind="ExternalInput"
    ).ap()
    out_ext = nc.dram_tensor("out", [rows, 64], F32, kind="ExternalOutput").ap()

    KW = npair * 128  # k columns per head
    VW = 2 * npair * 65  # v columns per head

    with tile.TileContext(nc) as tc:
        with (
            tc.tile_pool(name="persist", bufs=1) as persist,
            tc.tile_pool(name="stage", bufs=2) as stage,
            tc.tile_pool(name="pbuf", bufs=6) as pbuf,
            tc.tile_pool(name="work", bufs=2) as work,
            tc.tile_pool(name="qkpsum", bufs=2, space="PSUM") as qkpsum,
            tc.tile_pool(name="otpsum", bufs=2, space="PSUM") as otpsum,
        ):
            # ---- persistent operand tiles (split per head / per q-chunk
            # so the first segment's compute can start while later
            # heads are still loading) ----
            qc = min(2048, rows)
            nqc = rows // qc
            ident = persist.tile([65, 65], F32)
            make_identity(nc, ident[:])

            q8_t = [
                persist.tile([128, qc], F8, name=f"q8_{c}") for c in range(nqc)
            ]
            k8_t = [
                persist.tile([128, KW], F8, name=f"k8_{h}") for h in range(nheads)
            ]
            v8b_t = [
                persist.tile([128, VW], BF16, name=f"v8b_{h}")
                for h in range(nheads)
            ]

            def load_head(h):
                st = stage.tile([128, 2080], F32, tag="stage", name=f"stk{h}")
                nc.sync.dma_start(st[:, 0:KW], kp_ext[h])
                nc.any.tensor_copy(k8_t[h][:], st[:, 0:KW])
                st = stage.tile([128, 2080], F32, tag="stage", name=f"stv{h}")
                nc.sync.dma_start(st[:, 0:VW], vp_ext[h])
                v8f = work.tile([128, VW], F8, tag="v8f", name=f"v8f{h}")
                nc.any.tensor_copy(v8f[:], st[:, 0:VW])
                nc.any.tensor_copy(v8b_t[h][:], v8f[:])

            def load_q(c):
                st = stage.tile([128, 2080], F32, tag="stage", name=f"stq{c}")
                nc.sync.dma_start(st[0:64, 0:qc], qT_ext[:, ts(c, qc)])
                nc.sync.dma_start(st[64:128, 0:qc], qT_ext[:, ts(c, qc)])
                nc.any.tensor_copy(q8_t[c][:], st[:, 0:qc])

            load_head(0)
            load_q(0)
            for h in range(1, nheads):
                load_head(h)
            for c in range(1, nqc):
                load_q(c)

            # ---- main attention loops (software-pipelined) ----
            # Flat chunk list across segments: (head_slot, qtile, qo, qoff)
            chunks = []
            for slot, qbase, nq in segs:
                for chunk in range(nq // NQ):
                    qoff = qbase + chunk * NQ
                    chunks.append((slot, q8_t[qoff // qc], qoff % qc, qoff))

            def emit_qk_a(slot, qtile, qo, p):
                # QK^T row-packed pair: A on partitions 0-63, B on
                # 64-127 (tile_position auto-derived from base partition)
                qk = qkpsum.tile(
                    [128, 2 * NQ], F32, tag="qk", bufs=3, name="qk"
                )
                kA = k8_t[slot][0:64, p * 128 : (p + 1) * 128]
                nc.tensor.matmul(
                    qk[:, 0:NQ], kA, qtile[0:64, qo : qo + NQ],
                    start=True, stop=True,
                )
                return qk

            def emit_qk_b(slot, qtile, qo, p, qk):
                kB = k8_t[slot][64:128, p * 128 : (p + 1) * 128]
                nc.tensor.matmul(
                    qk[:, NQ : 2 * NQ], kB, qtile[64:128, qo : qo + NQ],
                    start=True, stop=True,
                )

            def emit_exp(qk, p):
                # exp of the whole pair tile [128, 2*NQ] in ONE op on one
                # engine (halves the fixed per-op overhead + sem count)
                pab = pbuf.tile([128, 2 * NQ], BF16, tag="p", name="pab")
                if p in dve_pairs:
                    nc.vector.tensor_scalar(
                        pab[:].bitcast(I16), qk[:],
                        SCH_A, SCH_B,
                        mybir.AluOpType.mult, mybir.AluOpType.add,
                    )
                else:
                    nc.scalar.activation(
                        pab[:], qk[:],
                        mybir.ActivationFunctionType.Exp, scale=0.125,
                    )
                return pab

            def emit_pv(slot, ot, pab, p, which):
                v = v8b_t[slot][
                    :, (2 * p + which) * 65 : (2 * p + which + 1) * 65
                ]
                nc.tensor.matmul(
                    ot[:], v, pab[:, which * NQ : (which + 1) * NQ],
                    start=(p == 0 and which == 0),
                    stop=(p == npair - 1 and which == 1),
                    skip_group_check=True,
                )

            def make_epilogue(ot, qoff):
                def epi():
                    ot_sb = work.tile([65, NQ], F32, tag="otsb", name="ot_sb")
                    nc.vector.tensor_copy(ot_sb[:], ot[:])
                    osb = work.tile([128, 4 * 64], F32, tag="osb", name="osb")
                    for t in range(4):
                        tr = otpsum.tile(
                            [128, 65], F32, tag="ot", bufs=2, name=f"tr{t}"
                        )
                        nc.tensor.transpose(tr[:], ot_sb[:, ts(t, 128)], ident[:])
                        rc = work.tile([128, 1], F32, tag="rc", name="rc")
                        nc.vector.reciprocal(rc[:], tr[:, 64:65])
                        nc.vector.tensor_scalar(
                            osb[:, ts(t, 64)], tr[:, 0:64],
                            rc[:], None, mybir.AluOpType.mult,
                        )
                    nc.sync.dma_start(
                        out_ext[qoff : qoff + NQ, :].rearrange(
                            "(b p) d -> p b d", p=128
                        ),
                        osb[:].rearrange("p (b d) -> p b d", d=64),
                    )

                return epi

            def emit_qk_pair(slot, qtile, qo, p):
                qk = emit_qk_a(slot, qtile, qo, p)
                emit_qk_b(slot, qtile, qo, p, qk)
                return qk

            pending_epi = None
            for slot, qtile, qo, qoff in chunks:
                ot = otpsum.tile([65, NQ], F32, tag="ot", bufs=2, name="ot")
                qks = {0: emit_qk_pair(slot, qtile, qo, 0)}
                if npair > 1:
                    qks[1] = emit_qk_pair(slot, qtile, qo, 1)
                for p in range(npair):
                    # keep the row-packed QK pair back-to-back (they
                    # overlap on the PE array), prefetched 2 pairs ahead
                    # so exp(p) completes before PV(p) reaches the queue
                    if p + 2 < npair:
                        qks[p + 2] = emit_qk_pair(slot, qtile, qo, p + 2)
                    if p == 1 and pending_epi is not None:
                        pending_epi()
                        pending_epi = None
                    pab = emit_exp(qks.pop(p), p)
                    emit_pv(slot, ot, pab, p, 0)
                    emit_pv(slot, ot, pab, p, 1)
                if pending_epi is not None:
                    pending_epi()
                pending_epi = make_epilogue(ot, qoff)
            pending_epi()

    nc.compile()
    return nc


def _prep_core_inputs(core, q, k, v):
    hA, hB, (hC, qoff) = _core_segments(core)
    qT = np.empty((64, ROWS_PER_CORE), np.float32)
    qT[:, 0:S] = q[hA].T
    qT[:, S : 2 * S] = q[hB].T
    qT[:, 2 * S :] = q[hC, qoff : qoff + HALF].T

    kp = np.empty((3, 128, NPAIR * 128), np.float32)
    vp = np.empty((3, 128, 32 * 65), np.float32)
    for slot, h in enumerate((hA, hB, hC)):
        kt = np.ascontiguousarray(k[h].T).reshape(64, 32, 128)
        kp[slot, 0:64] = kt[:, 0::2, :].reshape(64, NPAIR * 128)
        kp[slot, 64:128] = kt[:, 1::2, :].reshape(64, NPAIR * 128)
        vb = v[h].reshape(32, 128, 64).transpose(1, 0, 2)  # [128, 32, 64]
        vpk = np.concatenate(
            [vb, np.ones((128, 32, 1), np.float32)], axis=2
        )  # [128, 32, 65]
        vp[slot] = vpk.reshape(128, 32 * 65)
    return {"qT": np.ascontiguousarray(qT), "kp": kp, "vp": vp}


def kernel(q, k, v):
    global LAST_EXEC_TIME_NS, LAST_RESULTS, _CACHED
    q = np.asarray(q, np.float32)
    k = np.asarray(k, np.float32)
    v = np.asarray(v, np.float32)

    if _CACHED is None:
        _CACHED = _build_graph()
    nc = _CACHED

    in_maps = [_prep_core_inputs(i, q, k, v) for i in range(NCORES)]

    trace = os.environ.get("KERNEL_TRACE", "0") == "1"
    kwargs = {}
    if trace:
        kwargs = dict(trace=True, trace_cores=[0])
    res = run_bass_kernel_spmd(nc, in_maps, core_ids=list(range(NCORES)), **kwargs)
    LAST_RESULTS = res
    LAST_EXEC_TIME_NS = res.exec_time_ns

    out = np.empty((B, S, D), np.float32)
    for core in range(NCORES):
        o = res.results[core]["out"]
        hA, hB, (hC, qoff) = _core_segments(core)
        out[hA] = o[0:S]
        out[hB] = o[S : 2 * S]
        out[hC, qoff : qoff + HALF] = o[2 * S :]
    return out
